# revision 31
# baseline (speedup 1.0000x reference)
"""DCN block kernel v3 for Trainium2 (8 cores, data-parallel over batch).

Per core (one batch image, C=32 planes, 384x384):
  z = conv3x3(x, w_off)+b_off; s = sigmoid(z); d = s-.5 in (-.5,.5)
  sample at (r-dy, c-dx) bilinear w/ reflect  (|d|<.5 -> 3x3 support)
  y = conv3x3(sampled, w_dcn)+b_dcn

v3 structure (vs v2): fewer/larger ops everywhere.
  conv_off: per quarter-band u, 12 matmuls accumulate into one 4-bank
    PSUM tile; ONE sigmoid evacuates it to SB.  Fold SB -> interleaved
    quarter-stack fields SXX/SXY with 2 partition-rearranged DMAs.
  maps on ACT at FD3072: DX=.5s-.25, ADX=|.5s-.25|, M3=(PM,RM,QMn).
  stencils: AR/BRs on GpSimd (alignment-indifferent); BR in-place STT
    on DVE.
  sampling on DVE as 3-page tiles [128,3,8,W]: P3=DX*AR3, H3=X3+P3,
    Q3=ADX*BR3, H3+=Q3, T3=M3*H3, then S = H0 + Tm + Tp - T0 written
    into flat overlapping S-ring tiles [128, 10, WP] (rows r0-1..r0+8;
    boundary rows copied to neighbor ring slabs).
  conv_dcn (lagged 1 slab): OSR fold = 4 strided DMAs from the S ring
    into (4c+g)-partition layout; per u 12 col-paired matmuls into a
    2-bank PSUM tile, ONE bias-activation, ONE merged y store.
    Quarter-boundary pairs deferred to a cleanup pass (stashed rows).
"""

from contextlib import ExitStack

import ml_dtypes
import numpy as np

import concourse.bacc as bacc
import concourse.bass as bass
import concourse.mybir as mybir
import concourse.tile as tile

BF16 = mybir.dt.bfloat16
F32 = mybir.dt.float32
AF = mybir.ActivationFunctionType
OP = mybir.AluOpType

N_CORES = 8
C = 32
H = 384
W = 384
WP = W + 4  # data at cols 2..385, pads 0,1,386,387 (4B-aligned rows)
QH = H // 4       # 96 rows per quarter
NR = 8            # image rows per quarter per slab
NSLAB = QH // NR  # 12
NPAIR = NR // 2   # 4 row-pairs per band-slab


def build_nc(finalize=True):
    nc = bacc.Bacc()
    xr_in = nc.declare_dram_parameter("xr", [4, C, H // 2, WP], BF16, isOutput=False)
    xq_in = nc.declare_dram_parameter("xq", [4, C, QH + 4, WP], BF16, isOutput=False)
    xq2_in = nc.declare_dram_parameter("xq2", [4, C, QH + 4, WP], BF16, isOutput=False)
    woff_in = nc.declare_dram_parameter("woff", [128, 3 * 128], BF16, isOutput=False)
    wdcn_in = nc.declare_dram_parameter("wdcn", [128, 3 * 64], BF16, isOutput=False)
    boff_in = nc.declare_dram_parameter("boff", [128, 1], F32, isOutput=False)
    bdcn_in = nc.declare_dram_parameter("bdcn", [128, 1], F32, isOutput=False)
    # y[t, h, o, w] = out(plane o, row 2t+h, col w)
    y_out = nc.declare_dram_parameter("y", [H // 2, 2, C, W], BF16, isOutput=True)

    with tile.TileContext(nc) as tc, ExitStack() as ctx:
        consts = ctx.enter_context(tc.tile_pool(name="consts", bufs=1))
        xrpool = ctx.enter_context(tc.tile_pool(name="xrp", bufs=2))
        xspool = ctx.enter_context(tc.tile_pool(name="xsp", bufs=2))
        spool = ctx.enter_context(tc.tile_pool(name="sp", bufs=1))
        sxpool = ctx.enter_context(tc.tile_pool(name="sxp", bufs=1))
        mpool = ctx.enter_context(tc.tile_pool(name="mp", bufs=1))
        abpool = ctx.enter_context(tc.tile_pool(name="abp", bufs=1))
        wpool = ctx.enter_context(tc.tile_pool(name="wp", bufs=1))
        srpool = ctx.enter_context(tc.tile_pool(name="srp", bufs=1))
        osrpool = ctx.enter_context(tc.tile_pool(name="osrp", bufs=1))
        stpool = ctx.enter_context(tc.tile_pool(name="stp", bufs=2))
        zpool = ctx.enter_context(tc.tile_pool(name="zp", bufs=2, space="PSUM"))
        opool = ctx.enter_context(tc.tile_pool(name="op", bufs=2, space="PSUM"))

        WOFF = consts.tile([128, 3, 128], BF16)
        nc.sync.dma_start(out=WOFF[:].rearrange("p a b -> p (a b)"), in_=woff_in[:])
        WDCN = consts.tile([128, 3, 64], BF16)
        nc.sync.dma_start(out=WDCN[:].rearrange("p a b -> p (a b)"), in_=wdcn_in[:])
        WDCN2 = consts.tile([128, 3, 64], BF16)
        nc.sync.dma_start(out=WDCN2[:].rearrange("p a b -> p (a b)"), in_=wdcn_in[:])
        BOFF = consts.tile([128, 1], F32)
        nc.sync.dma_start(out=BOFF[:], in_=boff_in[:])
        BDCN = consts.tile([128, 1], F32)
        nc.sync.dma_start(out=BDCN[:], in_=bdcn_in[:])
        NQ = consts.tile([128, 1], F32)
        nc.vector.memset(NQ[:], -0.25)
        NH2 = consts.tile([128, 1], F32)
        nc.vector.memset(NH2[:], -0.5)
        PH2 = consts.tile([128, 1], F32)
        nc.vector.memset(PH2[:], 0.5)

        # Flat S ring: slab j at SR[j%3], slots 0..9 = image rows r0-1..r0+8
        # (slot 0 / slot 9 duplicated from neighbor slabs).  Pads pre-zeroed.
        SR = []
        for r in range(3):
            t = srpool.tile([128, NR + 2, WP], BF16, tag=f"sr{r}", name=f"sr{r}")
            nc.vector.memset(t[:].rearrange("p a b -> p (a b)"), 0.0)
            SR.append(t)
        # stash of rows 0..2 of every quarter (slab 0) for the cleanup pass
        ST3 = consts.tile([128, 3, WP], BF16)

        xr_q = xr_in.rearrange("g c (q p) w -> g c q p w", q=4)

        def sample_slab(it):
            r0 = it * NR
            b0 = r0 // 2
            # ---- loads (1 DMA each, DRAM-side partition merge) ----
            XR = xrpool.tile([128, 4, NPAIR, WP], BF16, tag="xr")
            nc.sync.dma_start(
                out=XR[:],
                in_=xr_q[:, :, :, b0:b0 + NPAIR, :].rearrange(
                    "g c u k w -> (g c) u k w"))
            XS = xspool.tile([128, NR + 2, WP], BF16, tag="xs")
            nc.sync.dma_start(
                out=XS[:],
                in_=xq_in[:, :, r0 + 1:r0 + 1 + NR + 2, :].rearrange(
                    "u c r w -> (u c) r w"))
            # col-shifted copy: XS2[., ., j] = X(col j-3); makes Xm/Xp aligned
            XS2 = xspool.tile([128, NR + 2, WP], BF16, tag="xs2")
            nc.sync.dma_start(
                out=XS2[:],
                in_=xq2_in[:, :, r0 + 1:r0 + 1 + NR + 2, :].rearrange(
                    "u c r w -> (u c) r w"))
            # ---- conv_off: per half-band, 6 weight-stationary matmuls ->
            #      2-bank PSUM -> sigmoid
            SB = spool.tile([128, 4, NPAIR, W], BF16, tag="sb")
            for u in range(4):
                for a in range(2):
                    ps = zpool.tile([128, 2, 512], F32, tag="z")
                    for kw in range(3):
                        for k2 in range(2):
                            nc.tensor.matmul(
                                ps[:, k2, 0:W],
                                lhsT=WOFF[:, kw, :],
                                rhs=XR[:, u, 2 * a + k2, kw + 1:kw + 1 + W],
                                start=(kw == 0), stop=(kw == 2))
                    nc.scalar.activation(out=SB[:, u, 2 * a:2 * a + 2, :],
                                         in_=ps[:, :, 0:W],
                                         func=AF.Sigmoid, bias=BOFF[:],
                                         scale=1.0)
            # ---- fold to paged quarter-stack fields [128, 2ax, 8, W] ----
            SXB = sxpool.tile([128, 2, NR, W], BF16, tag="sxb")
            for u in range(4):
                for h in range(2):
                    for ax in range(2):
                        nc.sync.dma_start(
                            out=SXB[32 * u:32 * u + 32, ax, h::2, :],
                            in_=SB[64 * h + 32 * ax:64 * h + 32 * ax + 32, u])
            # ---- maps on ACT; dpix = s - 0.5 ----
            # MP pages: 0=DXP, 1=DXN, 2=PM, 3=RM, 4=QMn (2:5 feeds T3 merged)
            MP = mpool.tile([128, 5, NR, W], BF16, tag="mp")
            nc.scalar.activation(out=MP[:, 0:3:2], in_=SXB[:], func=AF.Relu,
                                 bias=NH2[:], scale=1.0)
            nc.scalar.activation(out=MP[:, 1:5:3], in_=SXB[:], func=AF.Relu,
                                 bias=PH2[:], scale=-1.0)
            nc.scalar.activation(out=MP[:, 3], in_=SXB[:, 1], func=AF.Abs,
                                 bias=NH2[:], scale=1.0)
            DXP = MP[:, 0]
            DXN = MP[:, 1]
            # ---- stencils on DVE (aligned via XS2): DL=Xm-X, DR=Xp-X ----
            NH = NR + 2
            DL = abpool.tile([128, NH, W], BF16, tag="dl")
            DR = abpool.tile([128, NH, W], BF16, tag="dr")
            nc.vector.tensor_tensor(DL[:], XS2[:, :, 2:W + 2],
                                    XS[:, :, 2:W + 2], OP.subtract)
            nc.vector.tensor_tensor(DR[:], XS2[:, :, 4:W + 4],
                                    XS[:, :, 2:W + 2], OP.subtract)
            # reflect fixups at image cols 0 / W-1
            nc.vector.tensor_tensor(DL[:, :, 0], DL[:, :, 0],
                                    XS[:, :, 3], OP.add)
            nc.vector.tensor_tensor(DR[:, :, W - 1], DR[:, :, W - 1],
                                    XS[:, :, W], OP.add)
            # ---- 3-page sampling pipeline on DVE ----
            WK = wpool.tile([128, 3, NR, W], BF16, tag="wk", name=f"p3_{it}")
            for i in range(3):
                nc.vector.tensor_tensor(WK[:, i], DXP, DL[:, i:i + NR, :],
                                        OP.mult)
            H3 = wpool.tile([128, 3, NR, W], BF16, tag="h3", name=f"h3_{it}")
            for i in range(3):
                nc.vector.tensor_tensor(H3[:, i], WK[:, i],
                                        XS[:, i:i + NR, 2:W + 2], OP.add)
            WK = wpool.tile([128, 3, NR, W], BF16, tag="wk", name=f"q3_{it}")
            for i in range(3):
                nc.vector.tensor_tensor(WK[:, i], DXN, DR[:, i:i + NR, :],
                                        OP.mult)
            nc.vector.tensor_tensor(H3[:], H3[:], WK[:], OP.add)
            T3 = wpool.tile([128, 3, NR, W], BF16, tag="wk", name=f"t3_{it}")
            nc.vector.tensor_tensor(T3[:], MP[:, 2:5], H3[:], OP.mult)
            # ---- S = H0 + Tm + Tp - T0 into ring slots 1..8 ----
            cur = SR[it % 3]
            s_ = cur[:, 1:1 + NR, 2:W + 2]
            nc.vector.tensor_tensor(s_, T3[:, 0], H3[:, 1], OP.add)
            nc.vector.tensor_tensor(s_, s_, T3[:, 2], OP.add)
            nc.vector.tensor_tensor(s_, s_, T3[:, 1], OP.subtract)
            # reflect fixups at image rows 0 / H-1 (mirrored-corner term)
            if it == 0:
                FX = stpool.tile([128, W], BF16, tag="fx", name="fx0")
                nc.vector.tensor_tensor(FX[0:32], MP[0:32, 2, 0, :],
                                        H3[0:32, 2, 0, :], OP.mult)
                nc.vector.tensor_tensor(cur[0:32, 1, 2:W + 2],
                                        cur[0:32, 1, 2:W + 2], FX[0:32], OP.add)
            if it == NSLAB - 1:
                FX = stpool.tile([128, W], BF16, tag="fx", name="fx1")
                nc.vector.tensor_tensor(FX[96:128], MP[96:128, 4, NR - 1, :],
                                        H3[96:128, 0, NR - 1, :], OP.mult)
                nc.vector.tensor_tensor(cur[96:128, NR, 2:W + 2],
                                        cur[96:128, NR, 2:W + 2], FX[96:128],
                                        OP.add)
            # boundary rows to neighbor ring slabs
            if it > 0:
                nc.vector.tensor_copy(SR[(it - 1) % 3][:, NR + 1, :],
                                      cur[:, 1, :])
            if it < NSLAB - 1:
                nc.vector.tensor_copy(SR[(it + 1) % 3][:, 0, :],
                                      cur[:, NR, :])
            else:
                # slab 11 slot 9 = row 96 (beyond image) must be zero
                nc.vector.memset(cur[:, NR + 1, :], 0.0)
            if it == 0:
                nc.vector.tensor_copy(
                    ST3[:].rearrange("p a b -> p (a b)"),
                    cur[:, 1:4, :].rearrange("p a b -> p (a b)"))

        def dcn_store(u, jt, ps):
            """bias-activation + merged y store(s) for band u of slab jt."""
            ST = stpool.tile([128, 2, W], BF16, tag="st")
            nc.scalar.activation(out=ST[:], in_=ps[:, :, 0:W],
                                 func=AF.Identity, bias=BDCN[:], scale=1.0)
            tb = (QH * u + jt * NR) // 2
            for k in range(2):
                yv = y_out[tb + 2 * k:tb + 2 * k + 2].rearrange(
                    "v h c w -> (v h c) w")
                if jt == 0 and u >= 1 and k == 0:
                    # skip pair tb (v=0): cleanup pass covers it
                    nc.gpsimd.dma_start(
                        out=y_out[tb + 1].rearrange("h c w -> (h c) w"),
                        in_=ST[64:128, k])
                elif jt == NSLAB - 1 and u <= 2 and k == 1:
                    # skip pair tb+3 (v=1)
                    nc.gpsimd.dma_start(
                        out=y_out[tb + 2].rearrange("h c w -> (h c) w"),
                        in_=ST[0:64, k])
                else:
                    nc.gpsimd.dma_start(out=yv, in_=ST[:, k])

        def dcn_slab(jt):
            cur = SR[jt % 3]
            OSR = osrpool.tile([128, 4, NPAIR, WP], BF16, tag="osr")
            for g in range(4):
                eng = nc.gpsimd if g < 2 else nc.sync
                for u in range(4):
                    eng.dma_start(
                        out=OSR[32 * g:32 * g + 32, u],
                        in_=cur[32 * u:32 * u + 32, g:g + 2 * NPAIR - 1:2, :])
            for u in range(4):
                ps = opool.tile([128, 2, 512], F32, tag="o")
                for kw in range(3):
                    for v in range(2):
                        for ki, k in enumerate((0, 2)):
                            nc.tensor.matmul(
                                ps[64 * v:64 * v + 64, ki, 0:W],
                                lhsT=(WDCN if v == 0 else WDCN2)[:, kw, 0:64],
                                rhs=OSR[:, u, k + v, kw + 1:kw + 1 + W],
                                start=(kw == 0), stop=(kw == 2),
                                tile_position=(0, 64 * v),
                                skip_group_check=True)
                dcn_store(u, jt, ps)

        # dcn lags sampling by 2 slabs so its matmuls never wait on the
        # in-flight slab's DVE chain.  dcn is emitted BEFORE the sample:
        # sample(it) writes slot 0 of SR[(it+1)%3] == SR[(it-2)%3], which
        # dcn(it-2) must read first (program order fixes the WAR direction).
        for it in range(NSLAB + 2):
            if it >= 2:
                dcn_slab(it - 2)
            if it < NSLAB:
                sample_slab(it)

        # ---- cleanup: quarter-boundary pairs t = 48v-1 and 48v, v=1..3 ----
        last = SR[(NSLAB - 1) % 3]
        O2 = consts.tile([128, 3, 2, WP], BF16)
        for vq in range(3):  # quarter pair (vq, vq+1); output pairs 48(vq+1)-1,+0
            # pair A (=48v-1): window rows 96v-3..96v; g -> row 96v-3+g
            for g in range(3):
                nc.sync.dma_start(out=O2[32 * g:32 * g + 32, vq, 0, :],
                                  in_=last[32 * vq:32 * vq + 32, 6 + g, :])
            nc.sync.dma_start(out=O2[96:128, vq, 0, :],
                              in_=ST3[32 * vq + 32:32 * vq + 64, 0, :])
            # pair B (=48v): window rows 96v-1..96v+2; g -> row 96v-1+g
            nc.sync.dma_start(out=O2[0:32, vq, 1, :],
                              in_=last[32 * vq:32 * vq + 32, 8, :])
            for g in range(1, 4):
                nc.sync.dma_start(out=O2[32 * g:32 * g + 32, vq, 1, :],
                                  in_=ST3[32 * vq + 32:32 * vq + 64, g - 1, :])
        for v in range(1, 4):
            ps = opool.tile([128, 2, 512], F32, tag="o")
            for kw in range(3):
                for pr in range(2):
                    nc.tensor.matmul(
                        ps[64 * pr:64 * pr + 64, 0, 0:W],
                        lhsT=(WDCN if pr == 0 else WDCN2)[:, kw, 0:64],
                        rhs=O2[:, v - 1, pr, kw + 1:kw + 1 + W],
                        start=(kw == 0), stop=(kw == 2),
                        tile_position=(0, 64 * pr),
                        skip_group_check=True)
            ST = stpool.tile([128, 2, W], BF16, tag="st")
            nc.scalar.activation(out=ST[:, 0], in_=ps[:, 0, 0:W],
                                 func=AF.Identity, bias=BDCN[:], scale=1.0)
            tb = 48 * v - 1
            nc.sync.dma_start(
                out=y_out[tb:tb + 2].rearrange("v h c w -> (v h c) w"),
                in_=ST[:, 0])

    if finalize:
        nc.finalize()
    return nc


def prep_x(x_img):
    """Host-side packing for one core. x_img: [C, H, W] f32."""
    xb = np.asarray(x_img).astype(ml_dtypes.bfloat16)
    xpad = np.zeros((C, H + 2, WP), dtype=ml_dtypes.bfloat16)
    xpad[:, 1:H + 1, 2:W + 2] = xb
    xrs = np.ascontiguousarray(
        np.stack([xpad[:, g:g + H:2, :] for g in range(4)], axis=0))
    xpad2 = np.zeros((C, H + 4, WP), dtype=ml_dtypes.bfloat16)
    xpad2[:, 2:H + 2, 2:W + 2] = xb
    xqs = np.ascontiguousarray(
        np.stack([xpad2[:, QH * u:QH * u + QH + 4, :] for u in range(4)], axis=0))
    xpad3 = np.zeros((C, H + 4, WP), dtype=ml_dtypes.bfloat16)
    xpad3[:, 2:H + 2, 3:W + 3] = xb  # col-shifted: col j holds X(j-3)
    xq2s = np.ascontiguousarray(
        np.stack([xpad3[:, QH * u:QH * u + QH + 4, :] for u in range(4)], axis=0))
    return xrs, xqs, xq2s


def prep_weights(w_off, b_off, w_dcn, b_dcn):
    woff = np.zeros((128, 3, 128), dtype=np.float32)
    wdcn = np.zeros((128, 3, 64), dtype=np.float32)
    for g in range(4):
        for h in range(2):
            kh = g - h
            if 0 <= kh <= 2:
                for axis in range(2):
                    woff[32 * g:32 * g + 32, :,
                         64 * h + 32 * axis:64 * h + 32 * axis + 32] = \
                        w_off[axis::2, :, kh, :].transpose(1, 2, 0)
                wdcn[32 * g:32 * g + 32, :, 32 * h:32 * h + 32] = \
                    w_dcn[:, :, kh, :].transpose(1, 2, 0)
    boff = np.zeros((128, 1), np.float32)
    for h in range(2):
        for axis in range(2):
            boff[64 * h + 32 * axis:64 * h + 32 * axis + 32, 0] = b_off[axis::2]
    bdcn = np.zeros((128, 1), np.float32)
    for v in range(2):
        for h in range(2):
            bdcn[64 * v + 32 * h:64 * v + 32 * h + 32, 0] = b_dcn
    return {
        "woff": np.ascontiguousarray(
            woff.reshape(128, 3 * 128)).astype(ml_dtypes.bfloat16),
        "wdcn": np.ascontiguousarray(
            wdcn.reshape(128, 3 * 64)).astype(ml_dtypes.bfloat16),
        "boff": boff, "bdcn": bdcn,
    }


_NC_CACHE = {}


def _get_nc():
    if "nc" not in _NC_CACHE:
        _NC_CACHE["nc"] = build_nc()
    return _NC_CACHE["nc"]


def _run(x, w_off, b_off, w_dcn, b_dcn, **spmd_kwargs):
    from concourse.bass_utils import run_bass_kernel_spmd

    B = x.shape[0]
    assert x.shape == (B, C, H, W) and B == N_CORES
    nc = _get_nc()
    w = prep_weights(np.asarray(w_off, dtype=np.float32),
                     np.asarray(b_off, dtype=np.float32),
                     np.asarray(w_dcn, dtype=np.float32),
                     np.asarray(b_dcn, dtype=np.float32))
    in_maps = []
    xnp = np.asarray(x)
    for b in range(B):
        m = dict(w)
        m["xr"], m["xq"], m["xq2"] = prep_x(xnp[b])
        in_maps.append(m)
    return run_bass_kernel_spmd(nc, in_maps, list(range(N_CORES)), **spmd_kwargs)


def kernel(x, w_off, b_off, w_dcn, b_dcn):
    res = _run(x, w_off, b_off, w_dcn, b_dcn)
    outs = []
    for i in range(N_CORES):
        y = np.asarray(res.results[i]["y"]).astype(np.float32)  # [192,2,32,384]
        outs.append(y.reshape(H, C, W).transpose(1, 0, 2))
    return np.stack(outs, axis=0)


# revision 33
# speedup vs baseline: 1.0462x; 1.0462x over previous
"""DCN block kernel v3 for Trainium2 (8 cores, data-parallel over batch).

Per core (one batch image, C=32 planes, 384x384):
  z = conv3x3(x, w_off)+b_off; s = sigmoid(z); d = s-.5 in (-.5,.5)
  sample at (r-dy, c-dx) bilinear w/ reflect  (|d|<.5 -> 3x3 support)
  y = conv3x3(sampled, w_dcn)+b_dcn

v3 structure (vs v2): fewer/larger ops everywhere.
  conv_off: per quarter-band u, 12 matmuls accumulate into one 4-bank
    PSUM tile; ONE sigmoid evacuates it to SB.  Fold SB -> interleaved
    quarter-stack fields SXX/SXY with 2 partition-rearranged DMAs.
  maps on ACT at FD3072: DX=.5s-.25, ADX=|.5s-.25|, M3=(PM,RM,QMn).
  stencils: AR/BRs on GpSimd (alignment-indifferent); BR in-place STT
    on DVE.
  sampling on DVE as 3-page tiles [128,3,8,W]: P3=DX*AR3, H3=X3+P3,
    Q3=ADX*BR3, H3+=Q3, T3=M3*H3, then S = H0 + Tm + Tp - T0 written
    into flat overlapping S-ring tiles [128, 10, WP] (rows r0-1..r0+8;
    boundary rows copied to neighbor ring slabs).
  conv_dcn (lagged 1 slab): OSR fold = 4 strided DMAs from the S ring
    into (4c+g)-partition layout; per u 12 col-paired matmuls into a
    2-bank PSUM tile, ONE bias-activation, ONE merged y store.
    Quarter-boundary pairs deferred to a cleanup pass (stashed rows).
"""

from contextlib import ExitStack

import ml_dtypes
import numpy as np

import concourse.bacc as bacc
import concourse.bass as bass
import concourse.mybir as mybir
import concourse.tile as tile

BF16 = mybir.dt.bfloat16
F32 = mybir.dt.float32
AF = mybir.ActivationFunctionType
OP = mybir.AluOpType

N_CORES = 8
C = 32
H = 384
W = 384
WP = W + 4  # data at cols 2..385, pads 0,1,386,387 (4B-aligned rows)
QH = H // 4       # 96 rows per quarter
NR = 8            # image rows per quarter per slab
NSLAB = QH // NR  # 12
NPAIR = NR // 2   # 4 row-pairs per band-slab


def build_nc(finalize=True):
    nc = bacc.Bacc()
    xr_in = nc.declare_dram_parameter("xr", [4, C, H // 2, WP], BF16, isOutput=False)
    xq_in = nc.declare_dram_parameter("xq", [4, C, QH + 4, WP], BF16, isOutput=False)
    xq2_in = nc.declare_dram_parameter("xq2", [4, C, QH + 4, WP], BF16, isOutput=False)
    woff_in = nc.declare_dram_parameter("woff", [128, 3 * 128], BF16, isOutput=False)
    wdcn_in = nc.declare_dram_parameter("wdcn", [128, 3 * 64], BF16, isOutput=False)
    boff_in = nc.declare_dram_parameter("boff", [128, 1], F32, isOutput=False)
    bdcn_in = nc.declare_dram_parameter("bdcn", [128, 1], F32, isOutput=False)
    # y[t, h, o, w] = out(plane o, row 2t+h, col w)
    y_out = nc.declare_dram_parameter("y", [H // 2, 2, C, W], BF16, isOutput=True)

    with tile.TileContext(nc) as tc, ExitStack() as ctx:
        consts = ctx.enter_context(tc.tile_pool(name="consts", bufs=1))
        xrpool = ctx.enter_context(tc.tile_pool(name="xrp", bufs=2))
        xspool = ctx.enter_context(tc.tile_pool(name="xsp", bufs=2))
        spool = ctx.enter_context(tc.tile_pool(name="sp", bufs=1))
        sxpool = ctx.enter_context(tc.tile_pool(name="sxp", bufs=1))
        mpool = ctx.enter_context(tc.tile_pool(name="mp", bufs=1))
        abpool = ctx.enter_context(tc.tile_pool(name="abp", bufs=1))
        wpool = ctx.enter_context(tc.tile_pool(name="wp", bufs=1))
        srpool = ctx.enter_context(tc.tile_pool(name="srp", bufs=1))
        osrpool = ctx.enter_context(tc.tile_pool(name="osrp", bufs=1))
        stpool = ctx.enter_context(tc.tile_pool(name="stp", bufs=2))
        zpool = ctx.enter_context(tc.tile_pool(name="zp", bufs=2, space="PSUM"))
        opool = ctx.enter_context(tc.tile_pool(name="op", bufs=2, space="PSUM"))

        WOFF = consts.tile([128, 3, 128], BF16)
        nc.sync.dma_start(out=WOFF[:].rearrange("p a b -> p (a b)"), in_=woff_in[:])
        WDCN = consts.tile([128, 3, 64], BF16)
        nc.sync.dma_start(out=WDCN[:].rearrange("p a b -> p (a b)"), in_=wdcn_in[:])
        WDCN2 = consts.tile([128, 3, 64], BF16)
        nc.sync.dma_start(out=WDCN2[:].rearrange("p a b -> p (a b)"), in_=wdcn_in[:])
        BOFF = consts.tile([128, 1], F32)
        nc.sync.dma_start(out=BOFF[:], in_=boff_in[:])
        BDCN = consts.tile([128, 1], F32)
        nc.sync.dma_start(out=BDCN[:], in_=bdcn_in[:])
        NQ = consts.tile([128, 1], F32)
        nc.vector.memset(NQ[:], -0.25)
        NH2 = consts.tile([128, 1], F32)
        nc.vector.memset(NH2[:], -0.5)
        PH2 = consts.tile([128, 1], F32)
        nc.vector.memset(PH2[:], 0.5)

        # Flat S ring: slab j at SR[j%3], slots 0..9 = image rows r0-1..r0+8
        # (slot 0 / slot 9 duplicated from neighbor slabs).  Pads pre-zeroed.
        SR = []
        for r in range(3):
            t = srpool.tile([128, NR + 2, WP], BF16, tag=f"sr{r}", name=f"sr{r}")
            nc.vector.memset(t[:].rearrange("p a b -> p (a b)"), 0.0)
            SR.append(t)
        # stash of rows 0..2 of every quarter (slab 0) for the cleanup pass
        ST3 = consts.tile([128, 3, WP], BF16)

        xr_q = xr_in.rearrange("g c (q p) w -> g c q p w", q=4)

        def sample_slab(it):
            r0 = it * NR
            b0 = r0 // 2
            # ---- loads (1 DMA each, DRAM-side partition merge) ----
            XR = xrpool.tile([128, 4, NPAIR, WP], BF16, tag="xr")
            nc.sync.dma_start(
                out=XR[:],
                in_=xr_q[:, :, :, b0:b0 + NPAIR, :].rearrange(
                    "g c u k w -> (g c) u k w"))
            XS = xspool.tile([128, NR + 2, WP], BF16, tag="xs")
            nc.sync.dma_start(
                out=XS[:],
                in_=xq_in[:, :, r0 + 1:r0 + 1 + NR + 2, :].rearrange(
                    "u c r w -> (u c) r w"))
            # col-shifted copy: XS2[., ., j] = X(col j-3); makes Xm/Xp aligned
            XS2 = xspool.tile([128, NR + 2, WP], BF16, tag="xs2")
            nc.sync.dma_start(
                out=XS2[:],
                in_=xq2_in[:, :, r0 + 1:r0 + 1 + NR + 2, :].rearrange(
                    "u c r w -> (u c) r w"))
            # ---- conv_off: per half-band, 6 weight-stationary matmuls ->
            #      2-bank PSUM -> sigmoid
            SB = spool.tile([128, 4, NPAIR, W], BF16, tag="sb")
            for u in range(4):
                for a in range(2):
                    ps = zpool.tile([128, 2, 512], F32, tag="z")
                    for kw in range(3):
                        for k2 in range(2):
                            nc.tensor.matmul(
                                ps[:, k2, 0:W],
                                lhsT=WOFF[:, kw, :],
                                rhs=XR[:, u, 2 * a + k2, kw + 1:kw + 1 + W],
                                start=(kw == 0), stop=(kw == 2))
                    nc.scalar.activation(out=SB[:, u, 2 * a:2 * a + 2, :],
                                         in_=ps[:, :, 0:W],
                                         func=AF.Sigmoid, bias=BOFF[:],
                                         scale=1.0)
            # ---- fold to paged quarter-stack fields [128, 2ax, 8, W] ----
            SXB = sxpool.tile([128, 2, NR, W], BF16, tag="sxb")
            for u in range(4):
                for h in range(2):
                    for ax in range(2):
                        nc.sync.dma_start(
                            out=SXB[32 * u:32 * u + 32, ax, h::2, :],
                            in_=SB[64 * h + 32 * ax:64 * h + 32 * ax + 32, u])
            # ---- maps on ACT; dpix = s - 0.5 ----
            # MP pages: 0=DXP, 1=PM, 2=DXN, 3=QMn, 4=RM
            MP = mpool.tile([128, 5, NR, W], BF16, tag="mp")
            nc.scalar.activation(out=MP[:, 0:2], in_=SXB[:], func=AF.Relu,
                                 bias=NH2[:], scale=1.0)
            nc.scalar.activation(out=MP[:, 2:4], in_=SXB[:], func=AF.Relu,
                                 bias=PH2[:], scale=-1.0)
            nc.scalar.activation(out=MP[:, 4], in_=SXB[:, 1], func=AF.Abs,
                                 bias=NH2[:], scale=1.0)
            DXP = MP[:, 0]
            DXN = MP[:, 2]
            # ---- stencils on DVE (aligned via XS2): DL=Xm-X, DR=Xp-X ----
            NH = NR + 2
            DL = abpool.tile([128, NH, W], BF16, tag="dl")
            DR = abpool.tile([128, NH, W], BF16, tag="dr")
            nc.vector.tensor_tensor(DL[:], XS2[:, :, 2:W + 2],
                                    XS[:, :, 2:W + 2], OP.subtract)
            nc.vector.tensor_tensor(DR[:], XS2[:, :, 4:W + 4],
                                    XS[:, :, 2:W + 2], OP.subtract)
            # reflect fixups at image cols 0 / W-1
            nc.vector.tensor_tensor(DL[:, :, 0], DL[:, :, 0],
                                    XS[:, :, 3], OP.add)
            nc.vector.tensor_tensor(DR[:, :, W - 1], DR[:, :, W - 1],
                                    XS[:, :, W], OP.add)
            # ---- 3-page sampling pipeline on DVE ----
            WK = wpool.tile([128, 3, NR, W], BF16, tag="wk", name=f"p3_{it}")
            for i in range(3):
                nc.vector.tensor_tensor(WK[:, i], DXP, DL[:, i:i + NR, :],
                                        OP.mult)
            H3 = wpool.tile([128, 3, NR, W], BF16, tag="h3", name=f"h3_{it}")
            for i in range(3):
                nc.vector.tensor_tensor(H3[:, i], WK[:, i],
                                        XS[:, i:i + NR, 2:W + 2], OP.add)
            WK = wpool.tile([128, 3, NR, W], BF16, tag="wk", name=f"q3_{it}")
            for i in range(3):
                nc.vector.tensor_tensor(WK[:, i], DXN, DR[:, i:i + NR, :],
                                        OP.mult)
            nc.vector.tensor_tensor(H3[:], H3[:], WK[:], OP.add)
            T3 = wpool.tile([128, 3, NR, W], BF16, tag="wk", name=f"t3_{it}")
            nc.vector.tensor_tensor(T3[:, 0], MP[:, 1], H3[:, 0], OP.mult)
            nc.vector.tensor_tensor(T3[:, 1], MP[:, 4], H3[:, 1], OP.mult)
            nc.vector.tensor_tensor(T3[:, 2], MP[:, 3], H3[:, 2], OP.mult)
            # ---- S = H0 + Tm + Tp - T0 into ring slots 1..8 ----
            cur = SR[it % 3]
            s_ = cur[:, 1:1 + NR, 2:W + 2]
            nc.vector.tensor_tensor(s_, T3[:, 0], H3[:, 1], OP.add)
            nc.vector.tensor_tensor(s_, s_, T3[:, 2], OP.add)
            nc.vector.tensor_tensor(s_, s_, T3[:, 1], OP.subtract)
            # reflect fixups at image rows 0 / H-1 (mirrored-corner term)
            if it == 0:
                FX = stpool.tile([128, W], BF16, tag="fx", name="fx0")
                nc.vector.tensor_tensor(FX[0:32], MP[0:32, 1, 0, :],
                                        H3[0:32, 2, 0, :], OP.mult)
                nc.vector.tensor_tensor(cur[0:32, 1, 2:W + 2],
                                        cur[0:32, 1, 2:W + 2], FX[0:32], OP.add)
            if it == NSLAB - 1:
                FX = stpool.tile([128, W], BF16, tag="fx", name="fx1")
                nc.vector.tensor_tensor(FX[96:128], MP[96:128, 3, NR - 1, :],
                                        H3[96:128, 0, NR - 1, :], OP.mult)
                nc.vector.tensor_tensor(cur[96:128, NR, 2:W + 2],
                                        cur[96:128, NR, 2:W + 2], FX[96:128],
                                        OP.add)
            # boundary rows to neighbor ring slabs
            if it > 0:
                nc.vector.tensor_copy(SR[(it - 1) % 3][:, NR + 1, :],
                                      cur[:, 1, :])
            if it < NSLAB - 1:
                nc.vector.tensor_copy(SR[(it + 1) % 3][:, 0, :],
                                      cur[:, NR, :])
            else:
                # slab 11 slot 9 = row 96 (beyond image) must be zero
                nc.vector.memset(cur[:, NR + 1, :], 0.0)
            if it == 0:
                nc.vector.tensor_copy(
                    ST3[:].rearrange("p a b -> p (a b)"),
                    cur[:, 1:4, :].rearrange("p a b -> p (a b)"))

        def dcn_store(u, jt, ps):
            """bias-activation + merged y store(s) for band u of slab jt."""
            ST = stpool.tile([128, 2, W], BF16, tag="st")
            nc.scalar.activation(out=ST[:], in_=ps[:, :, 0:W],
                                 func=AF.Identity, bias=BDCN[:], scale=1.0)
            tb = (QH * u + jt * NR) // 2
            for k in range(2):
                yv = y_out[tb + 2 * k:tb + 2 * k + 2].rearrange(
                    "v h c w -> (v h c) w")
                if jt == 0 and u >= 1 and k == 0:
                    # skip pair tb (v=0): cleanup pass covers it
                    nc.gpsimd.dma_start(
                        out=y_out[tb + 1].rearrange("h c w -> (h c) w"),
                        in_=ST[64:128, k])
                elif jt == NSLAB - 1 and u <= 2 and k == 1:
                    # skip pair tb+3 (v=1)
                    nc.gpsimd.dma_start(
                        out=y_out[tb + 2].rearrange("h c w -> (h c) w"),
                        in_=ST[0:64, k])
                else:
                    nc.gpsimd.dma_start(out=yv, in_=ST[:, k])

        def dcn_slab(jt):
            cur = SR[jt % 3]
            OSR = osrpool.tile([128, 4, NPAIR, WP], BF16, tag="osr")
            for g in range(4):
                # tail slabs run after all sampling: split fold across two
                # DGE queues so the epilogue's folds issue in parallel
                eng = nc.sync if (jt >= NSLAB - 2 and g >= 2) else nc.gpsimd
                for u in range(4):
                    eng.dma_start(
                        out=OSR[32 * g:32 * g + 32, u],
                        in_=cur[32 * u:32 * u + 32, g:g + 2 * NPAIR - 1:2, :])
            for u in range(4):
                ps = opool.tile([128, 2, 512], F32, tag="o")
                for kw in range(3):
                    for v in range(2):
                        for ki, k in enumerate((0, 2)):
                            nc.tensor.matmul(
                                ps[64 * v:64 * v + 64, ki, 0:W],
                                lhsT=(WDCN if v == 0 else WDCN2)[:, kw, 0:64],
                                rhs=OSR[:, u, k + v, kw + 1:kw + 1 + W],
                                start=(kw == 0), stop=(kw == 2),
                                tile_position=(0, 64 * v),
                                skip_group_check=True)
                dcn_store(u, jt, ps)

        # dcn lags sampling by 2 slabs so its matmuls never wait on the
        # in-flight slab's DVE chain.  dcn is emitted BEFORE the sample:
        # sample(it) writes slot 0 of SR[(it+1)%3] == SR[(it-2)%3], which
        # dcn(it-2) must read first (program order fixes the WAR direction).
        for it in range(NSLAB + 2):
            if it >= 2:
                dcn_slab(it - 2)
            if it < NSLAB:
                sample_slab(it)

        # ---- cleanup: quarter-boundary pairs t = 48v-1 and 48v, v=1..3 ----
        last = SR[(NSLAB - 1) % 3]
        O2 = consts.tile([128, 3, 2, WP], BF16)
        for vq in range(3):  # quarter pair (vq, vq+1); output pairs 48(vq+1)-1,+0
            # pair A (=48v-1): window rows 96v-3..96v; g -> row 96v-3+g
            for g in range(3):
                nc.sync.dma_start(out=O2[32 * g:32 * g + 32, vq, 0, :],
                                  in_=last[32 * vq:32 * vq + 32, 6 + g, :])
            nc.sync.dma_start(out=O2[96:128, vq, 0, :],
                              in_=ST3[32 * vq + 32:32 * vq + 64, 0, :])
            # pair B (=48v): window rows 96v-1..96v+2; g -> row 96v-1+g
            nc.sync.dma_start(out=O2[0:32, vq, 1, :],
                              in_=last[32 * vq:32 * vq + 32, 8, :])
            for g in range(1, 4):
                nc.sync.dma_start(out=O2[32 * g:32 * g + 32, vq, 1, :],
                                  in_=ST3[32 * vq + 32:32 * vq + 64, g - 1, :])
        for v in range(1, 4):
            ps = opool.tile([128, 2, 512], F32, tag="o")
            for kw in range(3):
                for pr in range(2):
                    nc.tensor.matmul(
                        ps[64 * pr:64 * pr + 64, 0, 0:W],
                        lhsT=(WDCN if pr == 0 else WDCN2)[:, kw, 0:64],
                        rhs=O2[:, v - 1, pr, kw + 1:kw + 1 + W],
                        start=(kw == 0), stop=(kw == 2),
                        tile_position=(0, 64 * pr),
                        skip_group_check=True)
            ST = stpool.tile([128, 2, W], BF16, tag="st")
            nc.scalar.activation(out=ST[:, 0], in_=ps[:, 0, 0:W],
                                 func=AF.Identity, bias=BDCN[:], scale=1.0)
            tb = 48 * v - 1
            nc.sync.dma_start(
                out=y_out[tb:tb + 2].rearrange("v h c w -> (v h c) w"),
                in_=ST[:, 0])

    if finalize:
        nc.finalize()
    return nc


def prep_x(x_img):
    """Host-side packing for one core. x_img: [C, H, W] f32."""
    xb = np.asarray(x_img).astype(ml_dtypes.bfloat16)
    xpad = np.zeros((C, H + 2, WP), dtype=ml_dtypes.bfloat16)
    xpad[:, 1:H + 1, 2:W + 2] = xb
    xrs = np.ascontiguousarray(
        np.stack([xpad[:, g:g + H:2, :] for g in range(4)], axis=0))
    xpad2 = np.zeros((C, H + 4, WP), dtype=ml_dtypes.bfloat16)
    xpad2[:, 2:H + 2, 2:W + 2] = xb
    xqs = np.ascontiguousarray(
        np.stack([xpad2[:, QH * u:QH * u + QH + 4, :] for u in range(4)], axis=0))
    xpad3 = np.zeros((C, H + 4, WP), dtype=ml_dtypes.bfloat16)
    xpad3[:, 2:H + 2, 3:W + 3] = xb  # col-shifted: col j holds X(j-3)
    xq2s = np.ascontiguousarray(
        np.stack([xpad3[:, QH * u:QH * u + QH + 4, :] for u in range(4)], axis=0))
    return xrs, xqs, xq2s


def prep_weights(w_off, b_off, w_dcn, b_dcn):
    woff = np.zeros((128, 3, 128), dtype=np.float32)
    wdcn = np.zeros((128, 3, 64), dtype=np.float32)
    for g in range(4):
        for h in range(2):
            kh = g - h
            if 0 <= kh <= 2:
                for axis in range(2):
                    woff[32 * g:32 * g + 32, :,
                         64 * h + 32 * axis:64 * h + 32 * axis + 32] = \
                        w_off[axis::2, :, kh, :].transpose(1, 2, 0)
                wdcn[32 * g:32 * g + 32, :, 32 * h:32 * h + 32] = \
                    w_dcn[:, :, kh, :].transpose(1, 2, 0)
    boff = np.zeros((128, 1), np.float32)
    for h in range(2):
        for axis in range(2):
            boff[64 * h + 32 * axis:64 * h + 32 * axis + 32, 0] = b_off[axis::2]
    bdcn = np.zeros((128, 1), np.float32)
    for v in range(2):
        for h in range(2):
            bdcn[64 * v + 32 * h:64 * v + 32 * h + 32, 0] = b_dcn
    return {
        "woff": np.ascontiguousarray(
            woff.reshape(128, 3 * 128)).astype(ml_dtypes.bfloat16),
        "wdcn": np.ascontiguousarray(
            wdcn.reshape(128, 3 * 64)).astype(ml_dtypes.bfloat16),
        "boff": boff, "bdcn": bdcn,
    }


_NC_CACHE = {}


def _get_nc():
    if "nc" not in _NC_CACHE:
        _NC_CACHE["nc"] = build_nc()
    return _NC_CACHE["nc"]


def _run(x, w_off, b_off, w_dcn, b_dcn, **spmd_kwargs):
    from concourse.bass_utils import run_bass_kernel_spmd

    B = x.shape[0]
    assert x.shape == (B, C, H, W) and B == N_CORES
    nc = _get_nc()
    w = prep_weights(np.asarray(w_off, dtype=np.float32),
                     np.asarray(b_off, dtype=np.float32),
                     np.asarray(w_dcn, dtype=np.float32),
                     np.asarray(b_dcn, dtype=np.float32))
    in_maps = []
    xnp = np.asarray(x)
    for b in range(B):
        m = dict(w)
        m["xr"], m["xq"], m["xq2"] = prep_x(xnp[b])
        in_maps.append(m)
    return run_bass_kernel_spmd(nc, in_maps, list(range(N_CORES)), **spmd_kwargs)


def kernel(x, w_off, b_off, w_dcn, b_dcn):
    res = _run(x, w_off, b_off, w_dcn, b_dcn)
    outs = []
    for i in range(N_CORES):
        y = np.asarray(res.results[i]["y"]).astype(np.float32)  # [192,2,32,384]
        outs.append(y.reshape(H, C, W).transpose(1, 0, 2))
    return np.stack(outs, axis=0)
